# revision 21
# baseline (speedup 1.0000x reference)
"""Trainium2 Bass kernel for MergedQKVParallelLinearWithLoRA.

Computes out = x @ W_qkv^T + b_qkv + per-token-LoRA, where each token t uses
adapter l_t = lora_indices[t]:
    shrink_s = x @ A_s[l_t]^T            (R=16 per slice s in {q,k,v})
    out[:, slice_s] += shrink_s @ B_s[l_t]^T

Strategy (8 NeuronCores, token-parallel):
  - Each core handles 1024 tokens, all 6144 output columns.
  - Host pre-transposes: xT [H, Tc] per core, wT [H, OUT], aT [H, 3*L*R],
    bT [L*R, OUT] (per-slice packed), plus a one-hot adapter mask expanded to
    [3*L*R, Tc] so LoRA becomes two dense matmuls (L=16 is small):
        shrinkT_all = aT^T @ x^T          [768, Tc]   (dense over adapters)
        shrinkT     = shrinkT_all * mask  (zero non-selected adapters)
        lora_out    = shrinkT_slice^T @ bT_slice  (accumulated in PSUM on top
                                                   of the base GEMM)
  - mm_dtype picks the matmul input dtype: "f32r" (fp32 data bitcast to
    float32r, full PE rate at N=512) or "bf16" (half DMA traffic).
  - structure "std": PSUM tiles are [token, out]; stationary operand is the
    x tile (reloaded every matmul).  structure "T": PSUM tiles are
    [out, token] (output transposed); stationary operand is the w tile,
    reused across 2 consecutive matmuls (token chunks), bias added via
    per-partition tensor_scalar; host re-transposes the result.
  - PSUM accumulates fp32; bias added during the PSUM->SBUF copy on DVE.
  - Bulk loads are split across the two HWDGE rings: xT + w stream on the
    SP ring; aT/mask/bT/bias on the ACT ring so phase 1 isn't queued behind
    the full xT transfer.
"""

import numpy as np

T = 8192
H = 4096
OUT_Q = 4096
OUT_KV = 1024
OUT = OUT_Q + 2 * OUT_KV  # 6144
L = 16
R = 16
LR3 = 3 * L * R  # 768
NCORES = 8
TC = T // NCORES  # 1024

MM_DTYPE = "f32r"    # "f32r" | "bf16"
OUT_DTYPE = "f32"    # "f32" | "bf16"
STRUCTURE = "std"    # "std" | "P" | "T"
EXPAND_F32 = False   # run LoRA-expand matmuls in f32r even when mm_dtype=bf16
WQ_SPLIT = False     # alternate phase-2 w loads across the SP/ACT HWDGE rings
X_SPLIT = False      # load xT in token-halves with aT interleaved (early phase-1 start)
W_BUFS = 6           # w prefetch tile-pool depth
X_TILES = True       # xT as NH chunk tiles (early start + cross-iter overlap)

_cache = {}


def _build(h, out_q, out_kv, tc_tokens, reps=1, timing_inputs=False,
           skip_lora=False, skip_main=False, mm_dtype=MM_DTYPE,
           structure=STRUCTURE, out_dtype=None, expand_f32=EXPAND_F32,
           wq_split=WQ_SPLIT, x_split=X_SPLIT, w_bufs=W_BUFS,
           x_tiles=X_TILES):
    """Build the per-core Bass program. All cores run the same NEFF (SPMD).

    reps > 1 wraps the whole body in a device-side For_i loop — used by the
    test harness to measure per-iteration HW time via wall-clock deltas.
    timing_inputs=True declares inputs as Internal DRAM (uninitialized, no
    host transfer) so wall-clock deltas are dominated by device exec time.
    """
    import concourse.bass as bass  # noqa: F401
    import concourse.mybir as mybir
    import concourse.tile as tile
    from concourse import bacc

    f32 = mybir.dt.float32
    if mm_dtype == "f32r":
        mmd = mybir.dt.float32r   # SBUF compute dtype
        dmad = f32                # DRAM storage dtype
    else:
        mmd = mybir.dt.bfloat16
        dmad = mybir.dt.bfloat16
    bf16 = mybir.dt.bfloat16
    if out_dtype is None:
        out_dtype = OUT_DTYPE
    outd = f32 if out_dtype == "f32" else bf16
    # dtype for the LoRA-expand matmuls (stationary shrT slices + moving bt)
    exp_f32r = expand_f32 or mm_dtype == "f32r"
    expd = mybir.dt.float32r if exp_f32r else mmd
    bt_dram_d = f32 if exp_f32r else dmad

    def ld_bt(eng, dst, src):
        if exp_f32r:
            eng.dma_start(dst, src.bitcast(expd))
        else:
            eng.dma_start(dst, src)

    def ld(eng, dst, src):
        """DMA a DRAM slice into an SBUF tile of the matmul dtype."""
        if mm_dtype == "f32r":
            eng.dma_start(dst, src.bitcast(mmd))
        else:
            eng.dma_start(dst, src)

    out_total = out_q + 2 * out_kv
    NH = h // 128          # contraction tiles
    NT = tc_tokens // 128  # token tiles (output partition dim)
    NOB = out_total // 512  # output column blocks
    NQB = out_q // 512      # q blocks
    NKB = out_kv // 512     # k blocks
    NC512 = tc_tokens // 512  # 512-token chunks for shrink
    NJ = LR3 // 128        # 6 lr tiles

    assert out_q % 512 == 0 and out_kv % 512 == 0 and tc_tokens % 512 == 0

    nc = bacc.Bacc(None, target_bir_lowering=False)

    in_kw = {} if timing_inputs else {"kind": "ExternalInput"}
    xT = nc.dram_tensor("xT", [h, tc_tokens], dmad, **in_kw)
    wT = nc.dram_tensor("wT", [h, out_total], dmad, **in_kw)
    aT = nc.dram_tensor("aT", [h, LR3], dmad, **in_kw)
    bT = nc.dram_tensor("bT", [2 * 128, out_total], bt_dram_d, **in_kw)
    maskT = nc.dram_tensor("maskT", [LR3, tc_tokens], f32, **in_kw)
    if structure in ("std", "P"):
        biasb = nc.dram_tensor("biasb", [128, out_total], f32, **in_kw)
        out_shape = [tc_tokens, out_total]
    else:
        # bias regrouped on host to [128, out_total//128] (partition = out%128)
        biasb = nc.dram_tensor("biasb", [128, out_total // 128], f32, **in_kw)
        out_shape = [out_total, tc_tokens]
    if timing_inputs:
        # keep the big result internal; expose only a tiny sink so per-call
        # host<->device transfer stays negligible for wall-delta timing
        out = nc.dram_tensor("out", out_shape, outd)
        sink = nc.dram_tensor("sink", [128, 512], outd, kind="ExternalOutput")
    else:
        out = nc.dram_tensor("out", out_shape, outd, kind="ExternalOutput")
        sink = None

    with tile.TileContext(nc) as tc:
        from contextlib import ExitStack

        with ExitStack() as ctx:
            xp = ctx.enter_context(tc.tile_pool(name="xp", bufs=(NH if x_tiles else 1)))
            sp = ctx.enter_context(tc.tile_pool(name="sp", bufs=1))
            pp = ctx.enter_context(tc.tile_pool(name="pp", bufs=8, space="PSUM"))
            atp = ctx.enter_context(tc.tile_pool(name="atp", bufs=4))
            mp = ctx.enter_context(tc.tile_pool(name="mp", bufs=2))
            wp = ctx.enter_context(tc.tile_pool(name="wp", bufs=w_bufs))
            btp = ctx.enter_context(tc.tile_pool(name="btp", bufs=3))
            bp2 = ctx.enter_context(tc.tile_pool(name="bp2", bufs=2))
            op = ctx.enter_context(tc.tile_pool(name="op", bufs=4))

            loop_ctx = tc.For_i(0, reps, 1) if reps > 1 else None
            if loop_ctx is not None:
                loop_ctx.__enter__()

            # Resident x^T (partition = h % 128) on the SP ring — either one
            # [128, NH, Tc] tile, or NH chunk tiles so each chunk's reload in
            # the next rep waits only on its own last reader.
            at_pre0 = {}
            if x_tiles:
                xts = [
                    xp.tile([128, tc_tokens], mmd, name=f"xt_{a}", tag="xt")
                    for a in range(NH)
                ]
                for a in range(NH):
                    ld(nc.sync, xts[a][:], xT[a * 128:(a + 1) * 128, :])
                    if not skip_lora:
                        # interleave the th=0 aT loads so phase 1 isn't queued
                        # behind the whole xT stream on the SP ring
                        at = atp.tile([128, LR3], mmd, name=f"at_0_{a}", tag="at")
                        ld(nc.sync, at, aT[a * 128:(a + 1) * 128, :])
                        at_pre0[(0, a)] = at

                def xs_ap(hh, sl):
                    return xts[hh][:, sl]
            else:
                xT_sb = xp.tile([128, NH, tc_tokens], mmd, name="xT_sb", tag="xT_sb")

                def xs_ap(hh, sl):
                    return xT_sb[:, hh, sl]
            at_pre = at_pre0
            if (not x_tiles) and x_split and not skip_lora:
                # token-halves: th=0 chunks first, each followed by its aT tile
                # so phase 1 starts as soon as the first chunks land
                for a in range(NH):
                    ld(nc.sync, xT_sb[:, a, 0:512], xT[a * 128:(a + 1) * 128, 0:512])
                    at = atp.tile([128, LR3], mmd, name=f"at_0_{a}", tag="at")
                    ld(nc.sync, at, aT[a * 128:(a + 1) * 128, :])
                    at_pre[(0, a)] = at
                for a in range(NH):
                    ld(nc.sync, xT_sb[:, a, 512:tc_tokens],
                       xT[a * 128:(a + 1) * 128, 512:tc_tokens])
            elif not x_tiles:
                for a in range(NH):
                    ld(nc.sync, xT_sb[:, a, :], xT[a * 128:(a + 1) * 128, :])
            # Resident masked shrink^T: [128, NJ, Tc]
            shrT = sp.tile([128, NJ, tc_tokens], expd, name="shrT", tag="shrT")

            # ---- Phase 1: LoRA shrink (dense over adapters) + mask ----
            for th in range(NC512 if not skip_lora else 0):
                tsl = slice(th * 512, (th + 1) * 512)
                ps = [
                    pp.tile([128, 512], f32, name=f"shps_{th}_{j}", tag="ps")
                    for j in range(NJ)
                ]
                for hh in range(NH):
                    if (th, hh) in at_pre:
                        at = at_pre[(th, hh)]
                    else:
                        at = atp.tile([128, LR3], mmd, name=f"at_{th}_{hh}", tag="at")
                        ld(nc.sync, at, aT[hh * 128:(hh + 1) * 128, :])
                    for j in range(NJ):
                        nc.tensor.matmul(
                            ps[j][:],
                            at[:, j * 128:(j + 1) * 128],
                            xs_ap(hh, tsl),
                            start=(hh == 0),
                            stop=(hh == NH - 1),
                        )
                for j in range(NJ):
                    m = mp.tile([128, 512], f32, name=f"m_{th}_{j}", tag="m")
                    nc.sync.dma_start(m, maskT[j * 128:(j + 1) * 128, tsl])
                    nc.vector.tensor_mul(shrT[:, j, tsl], ps[j][:], m[:])

            # ---- Phase 2: base GEMM + LoRA expand + bias ----
            if structure == "std":
                for ob in range(NOB if not skip_main else 0):
                    osl = slice(ob * 512, (ob + 1) * 512)
                    # which slice (q/k/v) this 512-col block belongs to
                    if ob < NQB:
                        jbase = 0
                    elif ob < NQB + NKB:
                        jbase = 2
                    else:
                        jbase = 4
                    ps = [
                        pp.tile([128, 512], f32, name=f"mps_{ob}_{t}", tag="ps")
                        for t in range(NT)
                    ]
                    for hh in range(NH):
                        w = wp.tile([128, 512], mmd, name=f"w_{ob}_{hh}", tag="w")
                        weng = nc.scalar if (wq_split and hh % 2 == 1) else nc.sync
                        ld(weng, w, wT[hh * 128:(hh + 1) * 128, osl])
                        for t in range(NT):
                            nc.tensor.matmul(
                                ps[t][:],
                                xs_ap(hh, slice(t * 128, (t + 1) * 128)),
                                w[:],
                                start=(hh == 0),
                                stop=(skip_lora and hh == NH - 1),
                            )
                    for jj in range(2 if not skip_lora else 0):
                        bt = btp.tile([128, 512], expd, name=f"bt_{ob}_{jj}", tag="bt")
                        ld_bt(nc.sync, bt, bT[jj * 128:(jj + 1) * 128, osl])
                        for t in range(NT):
                            nc.tensor.matmul(
                                ps[t][:],
                                shrT[:, jbase + jj, t * 128:(t + 1) * 128],
                                bt[:],
                                start=False,
                                stop=(jj == 1),
                            )
                    bias_t = bp2.tile([128, 512], f32, name=f"bias_{ob}", tag="bias")
                    nc.sync.dma_start(bias_t, biasb[:, osl])
                    for t in range(NT):
                        o = op.tile([128, 512], outd, name=f"o_{ob}_{t}", tag="o")
                        nc.vector.tensor_add(o[:], ps[t][:], bias_t[:])
                        # out-stores also ride the ACT HWDGE queue
                        nc.scalar.dma_start(out[t * 128:(t + 1) * 128, osl], o[:])
            elif structure == "P":
                # std output layout, but out-block PAIRS share each stationary
                # x tile (and each stationary shrT tile in the expand), so the
                # PE stream is LDW : MM = 1 : 2.  Token dim processed in two
                # quads of 128-tiles (2 obs x 4 t = 8 PSUM banks in flight).
                # Cost: the w stream is read twice (once per token quad).
                assert NT == 8 and NOB % 2 == 0
                for g in range(NOB // 2 if not skip_main else 0):
                    ob0, ob1 = 2 * g, 2 * g + 1
                    osl0 = slice(ob0 * 512, (ob0 + 1) * 512)
                    osl1 = slice(ob1 * 512, (ob1 + 1) * 512)
                    if ob0 < NQB:
                        jbase = 0
                    elif ob0 < NQB + NKB:
                        jbase = 2
                    else:
                        jbase = 4
                    for tg in range(2):
                        ps0 = [
                            pp.tile([128, 512], f32, name=f"pps0_{g}_{tg}_{ti}", tag="ps")
                            for ti in range(4)
                        ]
                        ps1 = [
                            pp.tile([128, 512], f32, name=f"pps1_{g}_{tg}_{ti}", tag="ps")
                            for ti in range(4)
                        ]
                        for hh in range(NH):
                            w0 = wp.tile([128, 512], mmd, name=f"w0_{g}_{tg}_{hh}", tag="w")
                            ld(nc.sync, w0, wT[hh * 128:(hh + 1) * 128, osl0])
                            w1 = wp.tile([128, 512], mmd, name=f"w1_{g}_{tg}_{hh}", tag="w")
                            ld(nc.sync, w1, wT[hh * 128:(hh + 1) * 128, osl1])
                            for ti in range(4):
                                t = tg * 4 + ti
                                xs = xs_ap(hh, slice(t * 128, (t + 1) * 128))
                                nc.tensor.matmul(
                                    ps0[ti][:], xs, w0[:],
                                    start=(hh == 0),
                                    stop=(skip_lora and hh == NH - 1),
                                )
                                nc.tensor.matmul(
                                    ps1[ti][:], xs, w1[:],
                                    start=(hh == 0),
                                    stop=(skip_lora and hh == NH - 1),
                                )
                        for jj in range(2 if not skip_lora else 0):
                            bt0 = btp.tile([128, 512], expd, name=f"bt0_{g}_{tg}_{jj}", tag="bt")
                            ld_bt(nc.sync, bt0, bT[jj * 128:(jj + 1) * 128, osl0])
                            bt1 = btp.tile([128, 512], expd, name=f"bt1_{g}_{tg}_{jj}", tag="bt")
                            ld_bt(nc.sync, bt1, bT[jj * 128:(jj + 1) * 128, osl1])
                            for ti in range(4):
                                t = tg * 4 + ti
                                ss = shrT[:, jbase + jj, t * 128:(t + 1) * 128]
                                nc.tensor.matmul(
                                    ps0[ti][:], ss, bt0[:],
                                    start=False, stop=(jj == 1),
                                )
                                nc.tensor.matmul(
                                    ps1[ti][:], ss, bt1[:],
                                    start=False, stop=(jj == 1),
                                )
                        bias_t0 = bp2.tile([128, 512], f32, name=f"bias0_{g}_{tg}", tag="bias")
                        nc.sync.dma_start(bias_t0, biasb[:, osl0])
                        bias_t1 = bp2.tile([128, 512], f32, name=f"bias1_{g}_{tg}", tag="bias")
                        nc.sync.dma_start(bias_t1, biasb[:, osl1])
                        for ti in range(4):
                            t = tg * 4 + ti
                            o0 = op.tile([128, 512], outd, name=f"o0_{g}_{tg}_{ti}", tag="o")
                            nc.vector.tensor_add(o0[:], ps0[ti][:], bias_t0[:])
                            nc.scalar.dma_start(out[t * 128:(t + 1) * 128, osl0], o0[:])
                            o1 = op.tile([128, 512], outd, name=f"o1_{g}_{tg}_{ti}", tag="o")
                            nc.vector.tensor_add(o1[:], ps1[ti][:], bias_t1[:])
                            nc.scalar.dma_start(out[t * 128:(t + 1) * 128, osl1], o1[:])
            else:
                # Transposed-output structure: psum = [out128, tok512];
                # stationary w[:, k*128] reused for the 2 token chunks.
                assert tc_tokens == 1024, "T structure assumes 2 token chunks"
                bias_sb = bp2.tile(
                    [128, out_total // 128], f32, name="bias_sb", tag="bias"
                )
                nc.scalar.dma_start(bias_sb[:], biasb[:, :])
                for ob in range(NOB if not skip_main else 0):
                    osl = slice(ob * 512, (ob + 1) * 512)
                    if ob < NQB:
                        jbase = 0
                    elif ob < NQB + NKB:
                        jbase = 2
                    else:
                        jbase = 4
                    ps = [
                        pp.tile([128, 512], f32, name=f"tps_{ob}_{kc}", tag="ps")
                        for kc in range(8)
                    ]
                    for hh in range(NH):
                        w = wp.tile([128, 512], mmd, name=f"w_{ob}_{hh}", tag="w")
                        ld(nc.sync, w, wT[hh * 128:(hh + 1) * 128, osl])
                        for k in range(4):
                            for c in range(2):
                                nc.tensor.matmul(
                                    ps[k * 2 + c][:],
                                    w[:, k * 128:(k + 1) * 128],
                                    xs_ap(hh, slice(c * 512, (c + 1) * 512)),
                                    start=(hh == 0),
                                    stop=(skip_lora and hh == NH - 1),
                                )
                    if not skip_lora:
                        for jj in range(2):
                            bt = btp.tile(
                                [128, 512], expd, name=f"bt_{ob}_{jj}", tag="bt"
                            )
                            ld_bt(nc.scalar, bt, bT[jj * 128:(jj + 1) * 128, osl])
                            for k in range(4):
                                for c in range(2):
                                    nc.tensor.matmul(
                                        ps[k * 2 + c][:],
                                        bt[:, k * 128:(k + 1) * 128],
                                        shrT[:, jbase + jj, c * 512:(c + 1) * 512],
                                        start=False,
                                        stop=(jj == 1),
                                    )
                    for k in range(4):
                        og = ob * 4 + k
                        for c in range(2):
                            o = op.tile([128, 512], outd, name=f"o_{ob}_{k}_{c}", tag="o")
                            nc.vector.tensor_scalar_add(
                                o[:], ps[k * 2 + c][:], bias_sb[:, og:og + 1]
                            )
                            nc.scalar.dma_start(
                                out[og * 128:(og + 1) * 128, c * 512:(c + 1) * 512],
                                o[:],
                            )

            if loop_ctx is not None:
                loop_ctx.__exit__(None, None, None)

            if sink is not None:
                nc.scalar.dma_start(sink[:], out[0:128, 0:512])

    nc.compile()
    return nc


def _get_nc(h=H, out_q=OUT_Q, out_kv=OUT_KV, tc_tokens=TC, reps=1,
            timing_inputs=False, skip_lora=False, skip_main=False,
            mm_dtype=None, structure=None, out_dtype=None, expand_f32=None,
            wq_split=None, x_split=None, w_bufs=None, x_tiles=None):
    if mm_dtype is None:
        mm_dtype = MM_DTYPE
    if structure is None:
        structure = STRUCTURE
    if out_dtype is None:
        out_dtype = OUT_DTYPE
    if expand_f32 is None:
        expand_f32 = EXPAND_F32
    if wq_split is None:
        wq_split = WQ_SPLIT
    if x_split is None:
        x_split = X_SPLIT
    if w_bufs is None:
        w_bufs = W_BUFS
    if x_tiles is None:
        x_tiles = X_TILES
    key = (h, out_q, out_kv, tc_tokens, reps, timing_inputs, skip_lora,
           skip_main, mm_dtype, structure, out_dtype, expand_f32, wq_split,
           x_split, w_bufs, x_tiles)
    if key not in _cache:
        _cache[key] = _build(
            h, out_q, out_kv, tc_tokens, reps=reps, timing_inputs=timing_inputs,
            skip_lora=skip_lora, skip_main=skip_main, mm_dtype=mm_dtype,
            structure=structure, out_dtype=out_dtype, expand_f32=expand_f32,
            wq_split=wq_split, x_split=x_split, w_bufs=w_bufs, x_tiles=x_tiles,
        )
    return _cache[key]


def _host_prep(x, w_qkv, b_qkv, a_q, a_k, a_v, b_q, b_k, b_v, lora_indices,
               n_cores=NCORES, mm_dtype=None, structure=None, expand_f32=None):
    """Build per-core input maps (host-side transposes/packing/cast)."""
    import ml_dtypes

    if mm_dtype is None:
        mm_dtype = MM_DTYPE
    if structure is None:
        structure = STRUCTURE
    if expand_f32 is None:
        expand_f32 = EXPAND_F32

    f = np.float32
    st = f if mm_dtype == "f32r" else ml_dtypes.bfloat16
    x = np.ascontiguousarray(np.asarray(x, f))
    t_total, h = x.shape
    tc_tokens = t_total // n_cores
    out_q = np.asarray(b_q).shape[1]
    out_kv = np.asarray(b_k).shape[1]
    out_total = out_q + 2 * out_kv

    wT = np.ascontiguousarray(np.asarray(w_qkv, f).T.astype(st))  # [H, OUT]
    l, r = np.asarray(a_q).shape[:2]
    aT = np.ascontiguousarray(
        np.concatenate(
            [np.asarray(a, f).reshape(l * r, h) for a in (a_q, a_k, a_v)], axis=0
        ).T.astype(st)
    )  # [H, 3*L*R]
    bt_st = f if (expand_f32 or mm_dtype == "f32r") else st
    bT = np.ascontiguousarray(
        np.concatenate(
            [
                np.asarray(b, f).transpose(0, 2, 1).reshape(l * r, -1)
                for b in (b_q, b_k, b_v)
            ],
            axis=1,
        ).astype(bt_st)
    )  # [L*R, OUT]
    bias = np.asarray(b_qkv, f)
    if structure in ("std", "P"):
        biasb = np.ascontiguousarray(np.broadcast_to(bias, (128, out_total)))
    else:
        # [128, OUT//128]: biasb[p, g] = bias[g*128 + p]
        biasb = np.ascontiguousarray(bias.reshape(out_total // 128, 128).T)

    li = np.asarray(lora_indices).astype(np.int64)
    oh = (li[:, None] == np.arange(l)[None, :]).astype(f)       # [T, L]
    mask_exp = np.repeat(oh, r, axis=1)                          # [T, L*R]
    maskT_full = np.ascontiguousarray(np.tile(mask_exp.T, (3, 1)))  # [3LR, T]

    xT_st = np.ascontiguousarray(x.T.astype(st))                 # [H, T]

    in_maps = []
    for c in range(n_cores):
        tsl = slice(c * tc_tokens, (c + 1) * tc_tokens)
        in_maps.append(
            {
                "xT": np.ascontiguousarray(xT_st[:, tsl]),
                "wT": wT,
                "aT": aT,
                "bT": bT,
                "maskT": np.ascontiguousarray(maskT_full[:, tsl]),
                "biasb": biasb,
            }
        )
    return in_maps


def kernel(x, w_qkv, b_qkv, a_q, a_k, a_v, b_q, b_k, b_v, lora_indices):
    from concourse.bass_utils import run_bass_kernel_spmd

    in_maps = _host_prep(
        x, w_qkv, b_qkv, a_q, a_k, a_v, b_q, b_k, b_v, lora_indices
    )
    nc = _get_nc()
    core_ids = list(range(NCORES))
    res = run_bass_kernel_spmd(nc, in_maps, core_ids)
    if STRUCTURE in ("std", "P"):
        return np.concatenate(
            [res.results[c]["out"].astype(np.float32) for c in core_ids], axis=0
        )
    return np.concatenate(
        [res.results[c]["out"].T.astype(np.float32) for c in core_ids], axis=0
    )


# revision 22
# speedup vs baseline: 1.2964x; 1.2964x over previous
"""Trainium2 Bass kernel for MergedQKVParallelLinearWithLoRA.

Computes out = x @ W_qkv^T + b_qkv + per-token-LoRA, where each token t uses
adapter l_t = lora_indices[t]:
    shrink_s = x @ A_s[l_t]^T            (R=16 per slice s in {q,k,v})
    out[:, slice_s] += shrink_s @ B_s[l_t]^T

Strategy (8 NeuronCores, token-parallel):
  - Each core handles 1024 tokens, all 6144 output columns.
  - Host pre-transposes: xT [H, Tc] per core, wT [H, OUT], aT [H, 3*L*R],
    bT [L*R, OUT] (per-slice packed), plus a one-hot adapter mask expanded to
    [3*L*R, Tc] so LoRA becomes two dense matmuls (L=16 is small):
        shrinkT_all = aT^T @ x^T          [768, Tc]   (dense over adapters)
        shrinkT     = shrinkT_all * mask  (zero non-selected adapters)
        lora_out    = shrinkT_slice^T @ bT_slice  (accumulated in PSUM on top
                                                   of the base GEMM)
  - mm_dtype picks the matmul input dtype: "f32r" (fp32 data bitcast to
    float32r, full PE rate at N=512) or "bf16" (half DMA traffic).
  - structure "std": PSUM tiles are [token, out]; stationary operand is the
    x tile (reloaded every matmul).  structure "T": PSUM tiles are
    [out, token] (output transposed); stationary operand is the w tile,
    reused across 2 consecutive matmuls (token chunks), bias added via
    per-partition tensor_scalar; host re-transposes the result.
  - PSUM accumulates fp32; bias added during the PSUM->SBUF copy on DVE.
  - Bulk loads are split across the two HWDGE rings: xT + w stream on the
    SP ring; aT/mask/bT/bias on the ACT ring so phase 1 isn't queued behind
    the full xT transfer.
"""

import numpy as np

T = 8192
H = 4096
OUT_Q = 4096
OUT_KV = 1024
OUT = OUT_Q + 2 * OUT_KV  # 6144
L = 16
R = 16
LR3 = 3 * L * R  # 768
NCORES = 8
TC = T // NCORES  # 1024

MM_DTYPE = "f32r"    # "f32r" | "bf16"
OUT_DTYPE = "f32"    # "f32" | "bf16"
STRUCTURE = "std"    # "std" | "P" | "T"
EXPAND_F32 = False   # run LoRA-expand matmuls in f32r even when mm_dtype=bf16
WQ_SPLIT = False     # alternate phase-2 w loads across the SP/ACT HWDGE rings
X_SPLIT = False      # load xT in token-halves with aT interleaved (early phase-1 start)
W_BUFS = 6           # w prefetch tile-pool depth
X_TILES = True       # xT as NH chunk tiles (early start + cross-iter overlap)
W_PACK = False       # host packs W partition-major; w DMAs fetch hh-PAIRS in one shot

_cache = {}


def _build(h, out_q, out_kv, tc_tokens, reps=1, timing_inputs=False,
           skip_lora=False, skip_main=False, mm_dtype=MM_DTYPE,
           structure=STRUCTURE, out_dtype=None, expand_f32=EXPAND_F32,
           wq_split=WQ_SPLIT, x_split=X_SPLIT, w_bufs=W_BUFS,
           x_tiles=X_TILES, w_pack=W_PACK):
    """Build the per-core Bass program. All cores run the same NEFF (SPMD).

    reps > 1 wraps the whole body in a device-side For_i loop — used by the
    test harness to measure per-iteration HW time via wall-clock deltas.
    timing_inputs=True declares inputs as Internal DRAM (uninitialized, no
    host transfer) so wall-clock deltas are dominated by device exec time.
    """
    import concourse.bass as bass  # noqa: F401
    import concourse.mybir as mybir
    import concourse.tile as tile
    from concourse import bacc

    f32 = mybir.dt.float32
    if mm_dtype == "f32r":
        mmd = mybir.dt.float32r   # SBUF compute dtype
        dmad = f32                # DRAM storage dtype
    else:
        mmd = mybir.dt.bfloat16
        dmad = mybir.dt.bfloat16
    bf16 = mybir.dt.bfloat16
    if out_dtype is None:
        out_dtype = OUT_DTYPE
    outd = f32 if out_dtype == "f32" else bf16
    # dtype for the LoRA-expand matmuls (stationary shrT slices + moving bt)
    exp_f32r = expand_f32 or mm_dtype == "f32r"
    expd = mybir.dt.float32r if exp_f32r else mmd
    bt_dram_d = f32 if exp_f32r else dmad

    def ld_bt(eng, dst, src):
        if exp_f32r:
            eng.dma_start(dst, src.bitcast(expd))
        else:
            eng.dma_start(dst, src)

    def ld(eng, dst, src):
        """DMA a DRAM slice into an SBUF tile of the matmul dtype."""
        if mm_dtype == "f32r":
            eng.dma_start(dst, src.bitcast(mmd))
        else:
            eng.dma_start(dst, src)

    out_total = out_q + 2 * out_kv
    NH = h // 128          # contraction tiles
    NT = tc_tokens // 128  # token tiles (output partition dim)
    NOB = out_total // 512  # output column blocks
    NQB = out_q // 512      # q blocks
    NKB = out_kv // 512     # k blocks
    NC512 = tc_tokens // 512  # 512-token chunks for shrink
    NJ = LR3 // 128        # 6 lr tiles

    assert out_q % 512 == 0 and out_kv % 512 == 0 and tc_tokens % 512 == 0

    nc = bacc.Bacc(None, target_bir_lowering=False)

    in_kw = {} if timing_inputs else {"kind": "ExternalInput"}
    xT = nc.dram_tensor("xT", [h, tc_tokens], dmad, **in_kw)
    if w_pack:
        # partition-major: wP[p, a, o] = W^T[a*128 + p, o]
        wT = nc.dram_tensor("wT", [128, h // 128, out_total], dmad, **in_kw)
    else:
        wT = nc.dram_tensor("wT", [h, out_total], dmad, **in_kw)
    aT = nc.dram_tensor("aT", [h, LR3], dmad, **in_kw)
    bT = nc.dram_tensor("bT", [2 * 128, out_total], bt_dram_d, **in_kw)
    maskT = nc.dram_tensor("maskT", [LR3, tc_tokens], f32, **in_kw)
    if structure in ("std", "P"):
        biasb = nc.dram_tensor("biasb", [128, out_total], f32, **in_kw)
        out_shape = [tc_tokens, out_total]
    else:
        # bias regrouped on host to [128, out_total//128] (partition = out%128)
        biasb = nc.dram_tensor("biasb", [128, out_total // 128], f32, **in_kw)
        out_shape = [out_total, tc_tokens]
    if timing_inputs:
        # keep the big result internal; expose only a tiny sink so per-call
        # host<->device transfer stays negligible for wall-delta timing
        out = nc.dram_tensor("out", out_shape, outd)
        sink = nc.dram_tensor("sink", [128, 512], outd, kind="ExternalOutput")
    else:
        out = nc.dram_tensor("out", out_shape, outd, kind="ExternalOutput")
        sink = None

    with tile.TileContext(nc) as tc:
        from contextlib import ExitStack

        with ExitStack() as ctx:
            xp = ctx.enter_context(tc.tile_pool(name="xp", bufs=(NH if x_tiles else 1)))
            sp = ctx.enter_context(tc.tile_pool(name="sp", bufs=1))
            pp = ctx.enter_context(tc.tile_pool(name="pp", bufs=8, space="PSUM"))
            atp = ctx.enter_context(tc.tile_pool(name="atp", bufs=4))
            mp = ctx.enter_context(tc.tile_pool(name="mp", bufs=2))
            wp = ctx.enter_context(tc.tile_pool(name="wp", bufs=w_bufs))
            btp = ctx.enter_context(tc.tile_pool(name="btp", bufs=3))
            bp2 = ctx.enter_context(tc.tile_pool(name="bp2", bufs=2))
            op = ctx.enter_context(tc.tile_pool(name="op", bufs=4))

            loop_ctx = tc.For_i(0, reps, 1) if reps > 1 else None
            if loop_ctx is not None:
                loop_ctx.__enter__()

            # Resident x^T (partition = h % 128) on the SP ring — either one
            # [128, NH, Tc] tile, or NH chunk tiles so each chunk's reload in
            # the next rep waits only on its own last reader.
            at_pre0 = {}
            if x_tiles:
                xts = [
                    xp.tile([128, tc_tokens], mmd, name=f"xt_{a}", tag="xt")
                    for a in range(NH)
                ]
                for a in range(NH):
                    ld(nc.sync, xts[a][:], xT[a * 128:(a + 1) * 128, :])
                    if not skip_lora:
                        # interleave the th=0 aT loads so phase 1 isn't queued
                        # behind the whole xT stream on the SP ring
                        at = atp.tile([128, LR3], mmd, name=f"at_0_{a}", tag="at")
                        ld(nc.sync, at, aT[a * 128:(a + 1) * 128, :])
                        at_pre0[(0, a)] = at

                def xs_ap(hh, sl):
                    return xts[hh][:, sl]
            else:
                xT_sb = xp.tile([128, NH, tc_tokens], mmd, name="xT_sb", tag="xT_sb")

                def xs_ap(hh, sl):
                    return xT_sb[:, hh, sl]
            at_pre = at_pre0
            if (not x_tiles) and x_split and not skip_lora:
                # token-halves: th=0 chunks first, each followed by its aT tile
                # so phase 1 starts as soon as the first chunks land
                for a in range(NH):
                    ld(nc.sync, xT_sb[:, a, 0:512], xT[a * 128:(a + 1) * 128, 0:512])
                    at = atp.tile([128, LR3], mmd, name=f"at_0_{a}", tag="at")
                    ld(nc.sync, at, aT[a * 128:(a + 1) * 128, :])
                    at_pre[(0, a)] = at
                for a in range(NH):
                    ld(nc.sync, xT_sb[:, a, 512:tc_tokens],
                       xT[a * 128:(a + 1) * 128, 512:tc_tokens])
            elif not x_tiles:
                for a in range(NH):
                    ld(nc.sync, xT_sb[:, a, :], xT[a * 128:(a + 1) * 128, :])
            # Resident masked shrink^T: [128, NJ, Tc]
            shrT = sp.tile([128, NJ, tc_tokens], expd, name="shrT", tag="shrT")

            # ---- Phase 1: LoRA shrink (dense over adapters) + mask ----
            for th in range(NC512 if not skip_lora else 0):
                tsl = slice(th * 512, (th + 1) * 512)
                ps = [
                    pp.tile([128, 512], f32, name=f"shps_{th}_{j}", tag="ps")
                    for j in range(NJ)
                ]
                for hh in range(NH):
                    if (th, hh) in at_pre:
                        at = at_pre[(th, hh)]
                    else:
                        at = atp.tile([128, LR3], mmd, name=f"at_{th}_{hh}", tag="at")
                        ld(nc.sync, at, aT[hh * 128:(hh + 1) * 128, :])
                    for j in range(NJ):
                        nc.tensor.matmul(
                            ps[j][:],
                            at[:, j * 128:(j + 1) * 128],
                            xs_ap(hh, tsl),
                            start=(hh == 0),
                            stop=(hh == NH - 1),
                        )
                for j in range(NJ):
                    m = mp.tile([128, 512], f32, name=f"m_{th}_{j}", tag="m")
                    nc.sync.dma_start(m, maskT[j * 128:(j + 1) * 128, tsl])
                    nc.vector.tensor_mul(shrT[:, j, tsl], ps[j][:], m[:])

            # ---- Phase 2: base GEMM + LoRA expand + bias ----
            if structure == "std":
                for ob in range(NOB if not skip_main else 0):
                    osl = slice(ob * 512, (ob + 1) * 512)
                    # which slice (q/k/v) this 512-col block belongs to
                    if ob < NQB:
                        jbase = 0
                    elif ob < NQB + NKB:
                        jbase = 2
                    else:
                        jbase = 4
                    ps = [
                        pp.tile([128, 512], f32, name=f"mps_{ob}_{t}", tag="ps")
                        for t in range(NT)
                    ]
                    if w_pack:
                        for hp in range(NH // 2):
                            w2 = wp.tile([128, 2, 512], mmd,
                                         name=f"w_{ob}_{hp}", tag="w")
                            ld(nc.sync, w2, wT[:, 2 * hp:2 * hp + 2, osl])
                            for j in range(2):
                                hh = 2 * hp + j
                                for t in range(NT):
                                    nc.tensor.matmul(
                                        ps[t][:],
                                        xs_ap(hh, slice(t * 128, (t + 1) * 128)),
                                        w2[:, j, :],
                                        start=(hh == 0),
                                        stop=(skip_lora and hh == NH - 1),
                                    )
                    else:
                        for hh in range(NH):
                            w = wp.tile([128, 512], mmd, name=f"w_{ob}_{hh}", tag="w")
                            weng = nc.scalar if (wq_split and hh % 2 == 1) else nc.sync
                            ld(weng, w, wT[hh * 128:(hh + 1) * 128, osl])
                            for t in range(NT):
                                nc.tensor.matmul(
                                    ps[t][:],
                                    xs_ap(hh, slice(t * 128, (t + 1) * 128)),
                                    w[:],
                                    start=(hh == 0),
                                    stop=(skip_lora and hh == NH - 1),
                                )
                    for jj in range(2 if not skip_lora else 0):
                        bt = btp.tile([128, 512], expd, name=f"bt_{ob}_{jj}", tag="bt")
                        ld_bt(nc.sync, bt, bT[jj * 128:(jj + 1) * 128, osl])
                        for t in range(NT):
                            nc.tensor.matmul(
                                ps[t][:],
                                shrT[:, jbase + jj, t * 128:(t + 1) * 128],
                                bt[:],
                                start=False,
                                stop=(jj == 1),
                            )
                    bias_t = bp2.tile([128, 512], f32, name=f"bias_{ob}", tag="bias")
                    nc.sync.dma_start(bias_t, biasb[:, osl])
                    for t in range(NT):
                        o = op.tile([128, 512], outd, name=f"o_{ob}_{t}", tag="o")
                        nc.vector.tensor_add(o[:], ps[t][:], bias_t[:])
                        # out-stores also ride the ACT HWDGE queue
                        nc.scalar.dma_start(out[t * 128:(t + 1) * 128, osl], o[:])
            elif structure == "P":
                # std output layout, but out-block PAIRS share each stationary
                # x tile (and each stationary shrT tile in the expand), so the
                # PE stream is LDW : MM = 1 : 2.  Token dim processed in two
                # quads of 128-tiles (2 obs x 4 t = 8 PSUM banks in flight).
                # Cost: the w stream is read twice (once per token quad).
                assert NT == 8 and NOB % 2 == 0
                for g in range(NOB // 2 if not skip_main else 0):
                    ob0, ob1 = 2 * g, 2 * g + 1
                    osl0 = slice(ob0 * 512, (ob0 + 1) * 512)
                    osl1 = slice(ob1 * 512, (ob1 + 1) * 512)
                    if ob0 < NQB:
                        jbase = 0
                    elif ob0 < NQB + NKB:
                        jbase = 2
                    else:
                        jbase = 4
                    for tg in range(2):
                        ps0 = [
                            pp.tile([128, 512], f32, name=f"pps0_{g}_{tg}_{ti}", tag="ps")
                            for ti in range(4)
                        ]
                        ps1 = [
                            pp.tile([128, 512], f32, name=f"pps1_{g}_{tg}_{ti}", tag="ps")
                            for ti in range(4)
                        ]
                        for hh in range(NH):
                            w0 = wp.tile([128, 512], mmd, name=f"w0_{g}_{tg}_{hh}", tag="w")
                            ld(nc.sync, w0, wT[hh * 128:(hh + 1) * 128, osl0])
                            w1 = wp.tile([128, 512], mmd, name=f"w1_{g}_{tg}_{hh}", tag="w")
                            ld(nc.sync, w1, wT[hh * 128:(hh + 1) * 128, osl1])
                            for ti in range(4):
                                t = tg * 4 + ti
                                xs = xs_ap(hh, slice(t * 128, (t + 1) * 128))
                                nc.tensor.matmul(
                                    ps0[ti][:], xs, w0[:],
                                    start=(hh == 0),
                                    stop=(skip_lora and hh == NH - 1),
                                )
                                nc.tensor.matmul(
                                    ps1[ti][:], xs, w1[:],
                                    start=(hh == 0),
                                    stop=(skip_lora and hh == NH - 1),
                                )
                        for jj in range(2 if not skip_lora else 0):
                            bt0 = btp.tile([128, 512], expd, name=f"bt0_{g}_{tg}_{jj}", tag="bt")
                            ld_bt(nc.sync, bt0, bT[jj * 128:(jj + 1) * 128, osl0])
                            bt1 = btp.tile([128, 512], expd, name=f"bt1_{g}_{tg}_{jj}", tag="bt")
                            ld_bt(nc.sync, bt1, bT[jj * 128:(jj + 1) * 128, osl1])
                            for ti in range(4):
                                t = tg * 4 + ti
                                ss = shrT[:, jbase + jj, t * 128:(t + 1) * 128]
                                nc.tensor.matmul(
                                    ps0[ti][:], ss, bt0[:],
                                    start=False, stop=(jj == 1),
                                )
                                nc.tensor.matmul(
                                    ps1[ti][:], ss, bt1[:],
                                    start=False, stop=(jj == 1),
                                )
                        bias_t0 = bp2.tile([128, 512], f32, name=f"bias0_{g}_{tg}", tag="bias")
                        nc.sync.dma_start(bias_t0, biasb[:, osl0])
                        bias_t1 = bp2.tile([128, 512], f32, name=f"bias1_{g}_{tg}", tag="bias")
                        nc.sync.dma_start(bias_t1, biasb[:, osl1])
                        for ti in range(4):
                            t = tg * 4 + ti
                            o0 = op.tile([128, 512], outd, name=f"o0_{g}_{tg}_{ti}", tag="o")
                            nc.vector.tensor_add(o0[:], ps0[ti][:], bias_t0[:])
                            nc.scalar.dma_start(out[t * 128:(t + 1) * 128, osl0], o0[:])
                            o1 = op.tile([128, 512], outd, name=f"o1_{g}_{tg}_{ti}", tag="o")
                            nc.vector.tensor_add(o1[:], ps1[ti][:], bias_t1[:])
                            nc.scalar.dma_start(out[t * 128:(t + 1) * 128, osl1], o1[:])
            else:
                # Transposed-output structure: psum = [out128, tok512];
                # stationary w[:, k*128] reused for the 2 token chunks.
                assert tc_tokens == 1024, "T structure assumes 2 token chunks"
                bias_sb = bp2.tile(
                    [128, out_total // 128], f32, name="bias_sb", tag="bias"
                )
                nc.scalar.dma_start(bias_sb[:], biasb[:, :])
                for ob in range(NOB if not skip_main else 0):
                    osl = slice(ob * 512, (ob + 1) * 512)
                    if ob < NQB:
                        jbase = 0
                    elif ob < NQB + NKB:
                        jbase = 2
                    else:
                        jbase = 4
                    ps = [
                        pp.tile([128, 512], f32, name=f"tps_{ob}_{kc}", tag="ps")
                        for kc in range(8)
                    ]
                    for hh in range(NH):
                        w = wp.tile([128, 512], mmd, name=f"w_{ob}_{hh}", tag="w")
                        ld(nc.sync, w, wT[hh * 128:(hh + 1) * 128, osl])
                        for k in range(4):
                            for c in range(2):
                                nc.tensor.matmul(
                                    ps[k * 2 + c][:],
                                    w[:, k * 128:(k + 1) * 128],
                                    xs_ap(hh, slice(c * 512, (c + 1) * 512)),
                                    start=(hh == 0),
                                    stop=(skip_lora and hh == NH - 1),
                                )
                    if not skip_lora:
                        for jj in range(2):
                            bt = btp.tile(
                                [128, 512], expd, name=f"bt_{ob}_{jj}", tag="bt"
                            )
                            ld_bt(nc.scalar, bt, bT[jj * 128:(jj + 1) * 128, osl])
                            for k in range(4):
                                for c in range(2):
                                    nc.tensor.matmul(
                                        ps[k * 2 + c][:],
                                        bt[:, k * 128:(k + 1) * 128],
                                        shrT[:, jbase + jj, c * 512:(c + 1) * 512],
                                        start=False,
                                        stop=(jj == 1),
                                    )
                    for k in range(4):
                        og = ob * 4 + k
                        for c in range(2):
                            o = op.tile([128, 512], outd, name=f"o_{ob}_{k}_{c}", tag="o")
                            nc.vector.tensor_scalar_add(
                                o[:], ps[k * 2 + c][:], bias_sb[:, og:og + 1]
                            )
                            nc.scalar.dma_start(
                                out[og * 128:(og + 1) * 128, c * 512:(c + 1) * 512],
                                o[:],
                            )

            if loop_ctx is not None:
                loop_ctx.__exit__(None, None, None)

            if sink is not None:
                nc.scalar.dma_start(sink[:], out[0:128, 0:512])

    nc.compile()
    return nc


def _get_nc(h=H, out_q=OUT_Q, out_kv=OUT_KV, tc_tokens=TC, reps=1,
            timing_inputs=False, skip_lora=False, skip_main=False,
            mm_dtype=None, structure=None, out_dtype=None, expand_f32=None,
            wq_split=None, x_split=None, w_bufs=None, x_tiles=None,
            w_pack=None):
    if mm_dtype is None:
        mm_dtype = MM_DTYPE
    if structure is None:
        structure = STRUCTURE
    if out_dtype is None:
        out_dtype = OUT_DTYPE
    if expand_f32 is None:
        expand_f32 = EXPAND_F32
    if w_pack is None:
        w_pack = W_PACK
    if wq_split is None:
        wq_split = WQ_SPLIT
    if x_split is None:
        x_split = X_SPLIT
    if w_bufs is None:
        w_bufs = W_BUFS
    if x_tiles is None:
        x_tiles = X_TILES
    if w_pack is None:
        w_pack = W_PACK
    key = (h, out_q, out_kv, tc_tokens, reps, timing_inputs, skip_lora,
           skip_main, mm_dtype, structure, out_dtype, expand_f32, wq_split,
           x_split, w_bufs, x_tiles, w_pack)
    if key not in _cache:
        _cache[key] = _build(
            h, out_q, out_kv, tc_tokens, reps=reps, timing_inputs=timing_inputs,
            skip_lora=skip_lora, skip_main=skip_main, mm_dtype=mm_dtype,
            structure=structure, out_dtype=out_dtype, expand_f32=expand_f32,
            wq_split=wq_split, x_split=x_split, w_bufs=w_bufs, x_tiles=x_tiles,
            w_pack=w_pack,
        )
    return _cache[key]


def _host_prep(x, w_qkv, b_qkv, a_q, a_k, a_v, b_q, b_k, b_v, lora_indices,
               n_cores=NCORES, mm_dtype=None, structure=None, expand_f32=None,
               w_pack=None):
    """Build per-core input maps (host-side transposes/packing/cast)."""
    import ml_dtypes

    if mm_dtype is None:
        mm_dtype = MM_DTYPE
    if structure is None:
        structure = STRUCTURE
    if expand_f32 is None:
        expand_f32 = EXPAND_F32
    if w_pack is None:
        w_pack = W_PACK

    f = np.float32
    st = f if mm_dtype == "f32r" else ml_dtypes.bfloat16
    x = np.ascontiguousarray(np.asarray(x, f))
    t_total, h = x.shape
    tc_tokens = t_total // n_cores
    out_q = np.asarray(b_q).shape[1]
    out_kv = np.asarray(b_k).shape[1]
    out_total = out_q + 2 * out_kv

    wT = np.ascontiguousarray(np.asarray(w_qkv, f).T.astype(st))  # [H, OUT]
    if w_pack:
        # [128, H//128, OUT]: wP[p, a, o] = wT[a*128 + p, o]
        wT = np.ascontiguousarray(
            wT.reshape(h // 128, 128, -1).transpose(1, 0, 2)
        )
    l, r = np.asarray(a_q).shape[:2]
    aT = np.ascontiguousarray(
        np.concatenate(
            [np.asarray(a, f).reshape(l * r, h) for a in (a_q, a_k, a_v)], axis=0
        ).T.astype(st)
    )  # [H, 3*L*R]
    bt_st = f if (expand_f32 or mm_dtype == "f32r") else st
    bT = np.ascontiguousarray(
        np.concatenate(
            [
                np.asarray(b, f).transpose(0, 2, 1).reshape(l * r, -1)
                for b in (b_q, b_k, b_v)
            ],
            axis=1,
        ).astype(bt_st)
    )  # [L*R, OUT]
    bias = np.asarray(b_qkv, f)
    if structure in ("std", "P"):
        biasb = np.ascontiguousarray(np.broadcast_to(bias, (128, out_total)))
    else:
        # [128, OUT//128]: biasb[p, g] = bias[g*128 + p]
        biasb = np.ascontiguousarray(bias.reshape(out_total // 128, 128).T)

    li = np.asarray(lora_indices).astype(np.int64)
    oh = (li[:, None] == np.arange(l)[None, :]).astype(f)       # [T, L]
    mask_exp = np.repeat(oh, r, axis=1)                          # [T, L*R]
    maskT_full = np.ascontiguousarray(np.tile(mask_exp.T, (3, 1)))  # [3LR, T]

    xT_st = np.ascontiguousarray(x.T.astype(st))                 # [H, T]

    in_maps = []
    for c in range(n_cores):
        tsl = slice(c * tc_tokens, (c + 1) * tc_tokens)
        in_maps.append(
            {
                "xT": np.ascontiguousarray(xT_st[:, tsl]),
                "wT": wT,
                "aT": aT,
                "bT": bT,
                "maskT": np.ascontiguousarray(maskT_full[:, tsl]),
                "biasb": biasb,
            }
        )
    return in_maps


def kernel(x, w_qkv, b_qkv, a_q, a_k, a_v, b_q, b_k, b_v, lora_indices):
    from concourse.bass_utils import run_bass_kernel_spmd

    in_maps = _host_prep(
        x, w_qkv, b_qkv, a_q, a_k, a_v, b_q, b_k, b_v, lora_indices
    )
    nc = _get_nc()
    core_ids = list(range(NCORES))
    res = run_bass_kernel_spmd(nc, in_maps, core_ids)
    if STRUCTURE in ("std", "P"):
        return np.concatenate(
            [res.results[c]["out"].astype(np.float32) for c in core_ids], axis=0
        )
    return np.concatenate(
        [res.results[c]["out"].T.astype(np.float32) for c in core_ids], axis=0
    )
